# revision 17
# baseline (speedup 1.0000x reference)
"""Distributed attention kernel for Trainium2 (8 NeuronCores).

Reference computation (B=2, N=2048, C=1024, H=16, D=64, ALPHA=0.5):
    qkv = x @ W_qkv -> q,k,v [B,H,N,D]
    attn = softmax(q @ k^T / sqrt(D))
    attn = 0.5*dm + 0.5*attn
    out  = (attn @ v).reshape(B,N,C) @ W_proj + b_proj

Sharding: 8 cores = 2 batches x 4 head-groups (4 heads each).
Each core computes its head-group's slice end-to-end, including a partial
projection (row-slice of W_proj); host sums the 4 partials per batch.

On-device layout strategy (per core):
  - x arrives transposed [C, N] so the C-contraction has C on partitions.
  - q,k are produced transposed [Dg=256, N] (head-dim on partitions).
  - scores are computed transposed: S^T[k',q] = k^T.T @ q^T, so softmax's
    exp runs on ScalarE straight out of PSUM and the sum-over-k' is folded
    into the attn@v matmul via a ones-column appended to v (lhsT=[m, 65]:
    row 64 of the PSUM accumulator receives sum_m e[m,q] = the softmax
    denominator) -- no extra reduction pass over the N^2 matrix.
  - max-subtraction is skipped: scores are ~N(0,1), |s| < ~8 << 88, so
    exp never overflows in fp32.
  - dm is pre-halved + transposed on host and accumulated through its own
    matmul stream with v as the stationary operand.
  - the ones column holds 2.0, so the accumulator row is 2r and the
    normalization constant 0.5/r is a plain reciprocal.
  - normalization (per q column) is applied after attn@v on the small
    [64, 512] output tiles; the row vector 0.5/r is broadcast across
    partitions via a DRAM-bounce DMA (0-step partition APs are only legal
    on the DRAM side), or via a tiny fp16 PE matmul on the final chunk
    where the PE is idle.
  - all matmul operands are fp16 (1 cycle/row at the PE like bf16, but
    10-bit mantissa); PSUM accumulation stays fp32.
  - scores for a head pair land in one [128,1024] PSUM tile so each exp
    covers two heads (halves ScalarE instruction overhead -- ScalarE exp
    over the N^2 scores is the second-busiest engine after the PE).
"""

import numpy as np

B, N, C, H, D = 2, 2048, 1024, 16, 64
NCORES = 8
HG = 4                # head-groups per batch
HPC = H // HG         # heads per core = 4
DG = HPC * D          # 256: head-group width
SCALE = D ** -0.5

KT = C // 128         # 8 contraction tiles for qkv/x
NQ = N // 512         # 4 q-chunks
MT = N // 128         # 16 m (key) tiles


def _build_program():
    import concourse.bass as bass
    import concourse.bacc as bacc
    import concourse.tile as tile
    from concourse import mybir
    from contextlib import ExitStack

    f32 = mybir.dt.float32
    Exp = mybir.ActivationFunctionType.Exp
    f16 = mybir.dt.float16

    nc = bacc.Bacc()
    xT = nc.declare_dram_parameter("xT", [C, N], f16, isOutput=False)
    wq = nc.declare_dram_parameter("wq", [C, DG], f16, isOutput=False)
    wk = nc.declare_dram_parameter("wk", [C, DG], f16, isOutput=False)
    wv = nc.declare_dram_parameter("wv", [C, DG], f16, isOutput=False)
    wp = nc.declare_dram_parameter("wp", [DG, C], f16, isOutput=False)
    dmt = nc.declare_dram_parameter("dmt", [N, N], f16, isOutput=False)
    pout = nc.declare_dram_parameter("pout", [C, N], f16, isOutput=True)

    with tile.TileContext(nc) as tc, ExitStack() as ctx:
        big = ctx.enter_context(tc.tile_pool(name="big", bufs=1))
        epool = ctx.enter_context(tc.tile_pool(name="epool", bufs=4))
        small = ctx.enter_context(tc.tile_pool(name="small", bufs=2))
        outp = ctx.enter_context(tc.tile_pool(name="outp", bufs=4))
        # PSUM: psS slot [128,1024] x2 (4 banks) + pe0/pe1 (2) + pd0 (1) = 7 banks
        psS = ctx.enter_context(tc.tile_pool(name="psS", bufs=2, space="PSUM"))
        psE = ctx.enter_context(tc.tile_pool(name="psE", bufs=1, space="PSUM"))
        psD = ctx.enter_context(tc.tile_pool(name="psD", bufs=2, space="PSUM"))

        xt = big.tile([128, KT, N], f16)
        wq_s = big.tile([128, KT, DG], f16)
        wk_s = big.tile([128, KT, DG], f16)
        wv_s = big.tile([128, KT, DG], f16)
        qt = big.tile([128, 2, N], f16)
        kt = big.tile([128, 2, N], f16)
        vaug = big.tile([128, MT, HPC, D + 1], f16)
        vb = big.tile([128, MT, DG], f16)
        outT = big.tile([128, 2, N], f16)
        wp_s = big.tile([128, 2, C], f16)
        ones_sb = big.tile([128, MT * HPC], f32)
        ones16 = big.tile([1, D], f16)
        dms = big.tile([128, MT, N], f16)
        rscratch = nc.dram_tensor("rscratch", [8, 1024], f32)

        nc.vector.memset(ones_sb[:, :], 2.0)
        nc.vector.memset(ones16[:, :], 1.0)
        nc.vector.tensor_copy(vaug[:, :, :, D], ones_sb[:, :])

        for ct in range(KT):
            nc.sync.dma_start(out=xt[:, ct, :], in_=xT[ct * 128:(ct + 1) * 128, :])
            nc.sync.dma_start(out=wk_s[:, ct, :], in_=wk[ct * 128:(ct + 1) * 128, :])
        for ct in range(KT):
            nc.sync.dma_start(out=wv_s[:, ct, :], in_=wv[ct * 128:(ct + 1) * 128, :])
        for ct in range(KT):
            nc.sync.dma_start(out=wq_s[:, ct, :], in_=wq[ct * 128:(ct + 1) * 128, :])
        for jo in range(2):
            nc.sync.dma_start(out=wp_s[:, jo, :], in_=wp[jo * 128:(jo + 1) * 128, :])
        for mt in range(MT):
            nc.sync.dma_start(out=dms[:, mt, :], in_=dmt[mt * 128:(mt + 1) * 128, :])

        # ---- phase 1: k^T first, then v, then q^T (attn consumers need k/v whole) ----
        def qk_proj(w_s, dst, scale, goff):
            for jo in range(2):
                for nq in range(NQ):
                    g = goff + jo * NQ + nq
                    ps = psS.tile([128, 512], f32, name="ps", tag="psS")
                    for i in range(KT):
                        ct = (g + i) % KT
                        nc.tensor.matmul(
                            ps[:, :],
                            lhsT=w_s[:, ct, jo * 128:(jo + 1) * 128],
                            rhs=xt[:, ct, nq * 512:(nq + 1) * 512],
                            start=(i == 0), stop=(i == KT - 1),
                        )
                    if scale != 1.0:
                        nc.vector.tensor_scalar_mul(
                            dst[:, jo, nq * 512:(nq + 1) * 512], ps[:, :], scale)
                    else:
                        nc.vector.tensor_copy(dst[:, jo, nq * 512:(nq + 1) * 512], ps[:, :])

        # k^T: first 6 output groups accumulate ct-outer across 6 PSUM slots so
        # each arriving xt tile feeds 6 matmuls (PE keeps pace with the DMA).
        kgroups = [(jo, nq) for jo in range(2) for nq in range(NQ)]
        ktags = ["psS", "psS", "pe0", "pe1", "pd0", "pd0"]
        kps = {}
        for i, g in enumerate(kgroups[:6]):
            if ktags[i] in ("pe0", "pe1"):
                kps[g] = psE.tile([128, 512], f32, name=f"kp{i}", tag=ktags[i])
            elif ktags[i] == "pd0":
                kps[g] = psD.tile([128, 512], f32, name=f"kp{i}", tag="pd0")
            else:
                kps[g] = psS.tile([128, 512], f32, name=f"kp{i}", tag="psS")
        for ct in range(KT):
            for jo, nq in kgroups[:6]:
                nc.tensor.matmul(
                    kps[(jo, nq)][:, :],
                    lhsT=wk_s[:, ct, jo * 128:(jo + 1) * 128],
                    rhs=xt[:, ct, nq * 512:(nq + 1) * 512],
                    start=(ct == 0), stop=(ct == KT - 1),
                )
        for jo, nq in kgroups[:6]:
            nc.vector.tensor_copy(kt[:, jo, nq * 512:(nq + 1) * 512], kps[(jo, nq)][:, :])
        for jo, nq in kgroups[6:]:
            ps = psS.tile([128, 512], f32, name="ps", tag="psS")
            for i in range(KT):
                ct = (nq + i) % KT
                nc.tensor.matmul(
                    ps[:, :],
                    lhsT=wk_s[:, ct, jo * 128:(jo + 1) * 128],
                    rhs=xt[:, ct, nq * 512:(nq + 1) * 512],
                    start=(i == 0), stop=(i == KT - 1),
                )
            nc.vector.tensor_copy(kt[:, jo, nq * 512:(nq + 1) * 512], ps[:, :])

        for mt in range(MT):
            ps = psE.tile([128, DG], f32, name="ps", tag=f"pe{mt % 2}", padded_shape=[128, 512])
            for i in range(KT):
                ct = (mt + i) % KT
                nc.tensor.matmul(
                    ps[:, :],
                    lhsT=xt[:, ct, mt * 128:(mt + 1) * 128],
                    rhs=wv_s[:, ct, :],
                    start=(i == 0), stop=(i == KT - 1),
                )
            nc.vector.tensor_copy(vaug[:, mt, :, 0:D], ps[:, :])
            nc.vector.tensor_copy(vb[:, mt, :], ps[:, :])

        qk_proj(wq_s, qt, SCALE, 4)

        # ---- phase 2: attention, 2 heads (one k/q partition tile) per pass ----
        def proj_group(nq, co):
            qsl = slice(nq * 512, (nq + 1) * 512)
            ps = psD.tile([128, 512], f32, name="ps", tag="pd0")
            for jo in range(2):
                nc.tensor.matmul(
                    ps[:, :],
                    lhsT=wp_s[:, jo, co * 128:(co + 1) * 128],
                    rhs=outT[:, jo, qsl],
                    start=(jo == 0), stop=(jo == 1),
                )
            so = outp.tile([128, 512], f16)
            nc.vector.tensor_copy(so[:, :], ps[:, :])
            nc.sync.dma_start(out=pout[co * 128:(co + 1) * 128, qsl], in_=so[:, :])

        pending_proj = None
        for nq in range(NQ):
            qsl = slice(nq * 512, (nq + 1) * 512)
            for hp in range(2):
                pe0 = psE.tile([D + 1, 512], f32, name="pe0", tag="pe0")
                pe1 = psE.tile([D + 1, 512], f32, name="pe1", tag="pe1")
                pd = psD.tile([128, 512], f32, name="pd", tag="pd0")
                for mt in range(MT):
                    msl = slice(mt * 128, (mt + 1) * 128)
                    nc.tensor.matmul(
                        pd[:, :],
                        lhsT=vb[:, mt, hp * 128:(hp + 1) * 128],
                        rhs=dms[:, mt, qsl],
                        start=(mt == 0), stop=(mt == MT - 1),
                    )
                    sps = psS.tile([128, 1024], f32, name="sps", tag="psS")
                    nc.tensor.matmul(
                        sps[:, 0:512],
                        lhsT=kt[0:D, hp, msl], rhs=qt[0:D, hp, qsl],
                        start=True, stop=True,
                    )
                    nc.tensor.matmul(
                        sps[:, 512:1024],
                        lhsT=kt[D:2 * D, hp, msl], rhs=qt[D:2 * D, hp, qsl],
                        start=True, stop=True,
                    )
                    et = epool.tile([128, 1024], f16)
                    nc.scalar.activation(et[:, :], sps[:, :], Exp)
                    nc.tensor.matmul(
                        pe0[:, :], lhsT=vaug[:, mt, 2 * hp, :], rhs=et[:, 0:512],
                        start=(mt == 0), stop=(mt == MT - 1),
                    )
                    nc.tensor.matmul(
                        pe1[:, :], lhsT=vaug[:, mt, 2 * hp + 1, :], rhs=et[:, 512:1024],
                        start=(mt == 0), stop=(mt == MT - 1),
                    )
                    if pending_proj is not None and hp == 0 and 1 <= mt <= 8:
                        proj_group(pending_proj, mt - 1)
                # epilogue: free PSUM banks with quick copies, then normalize
                # off the critical path (broadcast 0.5/r via DRAM bounce).
                pe_s0 = small.tile([D + 1, 512], f32, name="pe_s0", tag="pe_s0")
                nc.vector.tensor_copy(pe_s0[:, :], pe0[:, :])
                pe_s1 = small.tile([D + 1, 512], f32, name="pe_s1", tag="pe_s1")
                nc.vector.tensor_copy(pe_s1[:, :], pe1[:, :])
                pd_s = small.tile([128, 512], f32, name="pd_s", tag="pd_s")
                nc.vector.tensor_copy(pd_s[:, :], pd[:, :])
                slot = nq * 2 + hp
                last = (nq == NQ - 1)
                rec2 = small.tile([1, 1024], f16 if last else f32, name="rec2",
                                  tag="rec2l" if last else "rec2")
                for half, pes in ((0, pe_s0), (1, pe_s1)):
                    with nc.allow_low_precision(reason="0.5/r broadcast"):
                        nc.vector.reciprocal(
                            rec2[:, half * 512:(half + 1) * 512], pes[D:D + 1, :])
                if last:
                    # tail fast path: PE is idle and psS slots are free here --
                    # broadcast via fp16 matmul instead of the DRAM bounce.
                    bcs = psS.tile([D, 1024], f32, name="bcp", tag="psS",
                                   padded_shape=[128, 1024])
                    nc.tensor.matmul(bcs[:, 0:512], lhsT=ones16[:, :],
                                     rhs=rec2[:, 0:512], start=True, stop=True)
                    nc.tensor.matmul(bcs[:, 512:1024], lhsT=ones16[:, :],
                                     rhs=rec2[:, 512:1024], start=True, stop=True)
                else:
                    nc.sync.dma_start(out=rscratch[slot:slot + 1, :], in_=rec2[:, :])
                    row = rscratch[slot, :]
                    bc_ap = bass.AP(tensor=row.tensor, offset=row.offset,
                                    ap=[[0, D]] + list(row.ap))
                    bcs = small.tile([D, 1024], f32, name="bcs", tag="bcs")
                    nc.sync.dma_start(out=bcs[:, :], in_=bc_ap)
                for half, pes in ((0, pe_s0), (1, pe_s1)):
                    t1 = small.tile([128, 512], f32, name="t1", tag="t1")
                    nc.vector.tensor_mul(
                        t1[half * D:(half + 1) * D, :], pes[0:D, :],
                        bcs[:, half * 512:(half + 1) * 512])
                    nc.vector.tensor_add(
                        outT[half * D:(half + 1) * D, hp, qsl],
                        t1[half * D:(half + 1) * D, :],
                        pd_s[half * D:(half + 1) * D, :],
                    )
            pending_proj = nq
        for co in range(C // 128):
            proj_group(NQ - 1, co)
    nc.compile()
    return nc


_PROGRAM = None


def _get_program():
    global _PROGRAM
    if _PROGRAM is None:
        _PROGRAM = _build_program()
    return _PROGRAM


def _make_in_maps(x, distance_matrix, W_qkv, W_proj):
    in_maps = []
    for core in range(NCORES):
        b, hg = divmod(core, HG)
        sl = slice(hg * DG, (hg + 1) * DG)
        in_maps.append({
            "xT": np.ascontiguousarray(x[b].T).astype(np.float16),
            "wq": np.ascontiguousarray(W_qkv[:, sl]).astype(np.float16),
            "wk": np.ascontiguousarray(W_qkv[:, C + hg * DG:C + (hg + 1) * DG]).astype(np.float16),
            "wv": np.ascontiguousarray(W_qkv[:, 2 * C + hg * DG:2 * C + (hg + 1) * DG]).astype(np.float16),
            "wp": np.ascontiguousarray(W_proj[sl, :]).astype(np.float16),
            "dmt": np.ascontiguousarray(
                (0.5 * distance_matrix[b, 0].T).astype(np.float16)
            ),
        })
    return in_maps


def kernel(x, distance_matrix, W_qkv, W_proj, b_proj, _results_hook=None):
    from concourse.bass_utils import run_bass_kernel_spmd

    nc = _get_program()
    in_maps = _make_in_maps(x, distance_matrix, W_qkv, W_proj)
    res = run_bass_kernel_spmd(nc, in_maps, list(range(NCORES)))
    if _results_hook is not None:
        _results_hook(res)
    out = np.zeros((B, N, C), dtype=np.float32)
    for core in range(NCORES):
        b = core // HG
        out[b] += res.results[core]["pout"].T
    out += b_proj[None, None, :].astype(np.float32)
    return out

